# revision 4
# baseline (speedup 1.0000x reference)
"""EvaAttention on 8 Trainium2 NeuronCores — head-parallel tensor parallelism.

Per core c (heads 2c, 2c+1):
  - q/k/v projections with column-sliced weights, dh-major layout
    (qT/kT/vT : [128 = 2 heads x 64 dims, 2048 tokens]), fp32r matmuls.
  - Conditional RoPE applied via host-precomputed transposed sin/cos tables
    (identity rows at class-token positions) + a pair-swap permutation matmul.
  - Attention with transposed scores (sT [keys, queries]) so softmax's
    reduction lands on the PE: exp on ScalarE (no max subtraction — scores
    are bounded, fp32), softmax sums via a ones column appended to the V
    stationary operand, normalization after AV.
  - Row-sliced out-projection partial product; host sums the 8 partials
    (the tensor-parallel all-reduce, performed at unshard time) + b_proj.
"""

import numpy as np

N = 2048
C = 1024
H = 16
D = 64  # head dim
NCORES = 8
HPC = H // NCORES  # heads per core = 2
DH = HPC * D  # per-core channel slice = 128
NT = 512
NTILES = N // NT  # 4
MT = 128
MTILES = N // MT  # 16
KC = C // 128  # 8 contraction chunks

_PROG = None


def _build_program():
    import concourse.bacc as bacc
    import concourse.mybir as mybir
    import concourse.tile as tile

    F32R = mybir.dt.float32r
    F32 = mybir.dt.float32
    AF = mybir.ActivationFunctionType
    ALU = mybir.AluOpType

    nc = bacc.Bacc()

    xT = nc.dram_tensor("xT", [C, N], F32R, kind="ExternalInput")
    wq = nc.dram_tensor("wq", [C, DH], F32R, kind="ExternalInput")
    wk = nc.dram_tensor("wk", [C, DH], F32R, kind="ExternalInput")
    wv = nc.dram_tensor("wv", [C, DH], F32R, kind="ExternalInput")
    wp = nc.dram_tensor("wp", [DH, C], F32R, kind="ExternalInput")
    bq = nc.dram_tensor("bq", [DH, 1], F32, kind="ExternalInput")
    bv = nc.dram_tensor("bv", [DH, 1], F32, kind="ExternalInput")
    cosq = nc.dram_tensor("cosq", [DH, N], F32, kind="ExternalInput")
    sinq = nc.dram_tensor("sinq", [DH, N], F32, kind="ExternalInput")
    cosk = nc.dram_tensor("cosk", [DH, N], F32, kind="ExternalInput")
    sink = nc.dram_tensor("sink", [DH, N], F32, kind="ExternalInput")
    perm = nc.dram_tensor("perm", [128, 128], F32R, kind="ExternalInput")
    ident = nc.dram_tensor("ident", [128, 128], F32R, kind="ExternalInput")
    ones64 = nc.dram_tensor("ones64", [1, D], F32R, kind="ExternalInput")
    vones = nc.dram_tensor("vones", [128, MTILES], F32R, kind="ExternalInput")
    out_part = nc.dram_tensor("out_part", [C, N], F32, kind="ExternalOutput")

    with tile.TileContext(nc) as tc, nc.allow_low_precision(reason="fp32r intermediates"):
        with (
            tc.tile_pool(name="const", bufs=1) as const,
            tc.tile_pool(name="work", bufs=1) as work,
        ):
            # ---- constant loads ----
            xts = []
            for kc in range(KC):
                t = const.tile([128, N], F32R, name=f"xts{kc}", tag=f"xts{kc}")
                nc.sync.dma_start(out=t, in_=xT[kc * 128 : (kc + 1) * 128, :])
                xts.append(t)
            wq_sb = const.tile([128, C], F32R)
            wk_sb = const.tile([128, C], F32R)
            wv_sb = const.tile([128, C], F32R)
            for kc in range(KC):
                sl = slice(kc * 128, (kc + 1) * 128)
                nc.sync.dma_start(out=wq_sb[:, sl], in_=wq[sl, :])
                nc.sync.dma_start(out=wk_sb[:, sl], in_=wk[sl, :])
                nc.sync.dma_start(out=wv_sb[:, sl], in_=wv[sl, :])
            wp_sb = const.tile([128, C], F32R)
            nc.sync.dma_start(out=wp_sb, in_=wp[:, :])
            bq_t = const.tile([DH, 1], F32)
            bv_t = const.tile([DH, 1], F32)
            nc.sync.dma_start(out=bq_t, in_=bq[:, :])
            nc.sync.dma_start(out=bv_t, in_=bv[:, :])
            perm_t = const.tile([128, 128], F32R)
            ident_t = const.tile([128, 128], F32R)
            nc.sync.dma_start(out=perm_t, in_=perm[:, :])
            nc.sync.dma_start(out=ident_t, in_=ident[:, :])
            ones_t = const.tile([1, D], F32R)
            nc.sync.dma_start(out=ones_t, in_=ones64[:, :])
            vones_t = const.tile([128, MTILES], F32R)
            nc.sync.dma_start(out=vones_t, in_=vones[:, :])

            # ---- persistent work tiles ----
            qT = work.tile([DH, N], F32R)  # roped, pre-scaled by d^-0.5
            kT = work.tile([DH, N], F32R)  # roped
            vT = work.tile([DH, N], F32R)
            va = [work.tile([128, MTILES * (D + 1)], F32R, name=f"va{h}", tag=f"va{h}") for h in range(HPC)]
            va3 = [v.rearrange("p (t e) -> p t e", e=D + 1) for v in va]
            oT = work.tile([DH, N], F32R)  # normalized per-head outputs (+ b_v)

            # ---- phase 1: projections + rope ----
            with (
                tc.tile_pool(name="p1sb", bufs=2) as p1sb,
                tc.tile_pool(name="p1ps", bufs=2, space="PSUM") as p1ps,
            ):
                for nt in range(NTILES):
                    ntsl = slice(nt * NT, (nt + 1) * NT)
                    # v projection (no rope)
                    psv = p1ps.tile([128, NT], F32, tag="proj")
                    for kc in range(KC):
                        nc.tensor.matmul(
                            psv,
                            wv_sb[:, kc * 128 : (kc + 1) * 128],
                            xts[kc][:, ntsl],
                            start=(kc == 0),
                            stop=(kc == KC - 1),
                        )
                    nc.scalar.copy(vT[:, ntsl], psv)
                    # q and k projections + rope
                    for which in ("q", "k"):
                        ps = p1ps.tile([128, NT], F32, tag="proj", name=f"ps_{which}{nt}")
                        w_sb = wq_sb if which == "q" else wk_sb
                        for kc in range(KC):
                            nc.tensor.matmul(
                                ps,
                                w_sb[:, kc * 128 : (kc + 1) * 128],
                                xts[kc][:, ntsl],
                                start=(kc == 0),
                                stop=(kc == KC - 1),
                            )
                        raw = p1sb.tile([128, NT], F32R, tag="raw", name=f"raw_{which}{nt}")
                        if which == "q":
                            nc.scalar.activation(raw, ps, AF.Identity, bias=bq_t)
                        else:
                            nc.scalar.copy(raw, ps)
                        # pair-swap via permutation matmul
                        psw = p1ps.tile([128, NT], F32, tag="swap", name=f"psw_{which}{nt}")
                        nc.tensor.matmul(psw, perm_t, raw, start=True, stop=True)
                        cos_d = cosq if which == "q" else cosk
                        sin_d = sinq if which == "q" else sink
                        cs = p1sb.tile([128, NT], F32, tag="cs", name=f"cs_{which}{nt}")
                        sn = p1sb.tile([128, NT], F32, tag="sn", name=f"sn_{which}{nt}")
                        nc.sync.dma_start(out=cs, in_=cos_d[:, ntsl])
                        nc.sync.dma_start(out=sn, in_=sin_d[:, ntsl])
                        t1 = p1sb.tile([128, NT], F32, tag="t1", name=f"t1_{which}{nt}")
                        nc.vector.tensor_tensor(t1, psw, sn, ALU.mult)
                        dst = qT if which == "q" else kT
                        nc.vector.tensor_tensor(dst[:, ntsl], raw, cs, ALU.mult)
                        nc.vector.tensor_tensor(dst[:, ntsl], dst[:, ntsl], t1, ALU.add)

            # ---- phase 2: transpose v to token-major, append ones column ----
            for h in range(HPC):
                nc.sync.dma_start(out=va3[h][:, :, D], in_=vones_t[:, :])
            with tc.tile_pool(name="p2ps", bufs=2, space="PSUM") as p2ps:
                for mt in range(MTILES):
                    pst = p2ps.tile([128, 128], F32R, tag="tr")
                    nc.tensor.transpose(pst, vT[:, mt * 128 : (mt + 1) * 128], ident_t)
                    for h in range(HPC):
                        nc.vector.tensor_copy(va3[h][:, mt, 0:D], pst[:, h * D : (h + 1) * D])

            # ---- phase 3: attention (per head) ----
            with (
                tc.tile_pool(name="p3ps", bufs=1, space="PSUM") as p3ps,
                tc.tile_pool(name="p3s", bufs=2, space="PSUM") as p3s,
                tc.tile_pool(name="p3sb", bufs=3) as p3sb,
                tc.tile_pool(name="p3misc", bufs=1) as p3misc,
            ):
                for h in range(HPC):
                    hsl = slice(h * D, (h + 1) * D)
                    o_ps = p3ps.tile([D + 1, N], F32, tag="o", name=f"o_ps{h}")
                    for mt in range(MTILES):
                        mtsl = slice(mt * 128, (mt + 1) * 128)
                        for half in range(2):
                            ps_s = p3s.tile([128, 2 * NT], F32, tag="s", name=f"s{h}_{mt}_{half}")
                            for sub in range(2):
                                qsl = slice((half * 2 + sub) * NT, (half * 2 + sub + 1) * NT)
                                nc.tensor.matmul(
                                    ps_s[:, sub * NT : (sub + 1) * NT],
                                    kT[hsl, mtsl],
                                    qT[hsl, qsl],
                                    start=True,
                                    stop=True,
                                )
                            pT = p3sb.tile([128, 2 * NT], F32R, tag="p", name=f"p{h}_{mt}_{half}")
                            nc.scalar.activation(pT, ps_s, AF.Exp)
                            for sub in range(2):
                                cols = slice((half * 2 + sub) * NT, (half * 2 + sub + 1) * NT)
                                nc.tensor.matmul(
                                    o_ps[:, cols],
                                    va3[h][:, mt, :],
                                    pT[:, sub * NT : (sub + 1) * NT],
                                    start=(mt == 0),
                                    stop=(mt == MTILES - 1),
                                )
                    # normalize: rows 0..63 are o, row 64 is the softmax sum
                    rc = p3misc.tile([1, N], F32R, name=f"rc{h}", tag=f"rc{h}")
                    nc.vector.reciprocal(rc, o_ps[D : D + 1, :])
                    o_sb = p3misc.tile([D, N], F32, name=f"osb{h}", tag=f"osb{h}")
                    nc.vector.tensor_copy(o_sb, o_ps[0:D, :])
                    for half in range(2):
                        csl = slice(half * 2 * NT, (half + 1) * 2 * NT)
                        rb = p3s.tile([D, 2 * NT], F32, tag="s", name=f"rb{h}_{half}")
                        for sub in range(2):
                            nc.tensor.matmul(
                                rb[:, sub * NT : (sub + 1) * NT],
                                ones_t,
                                rc[:, (half * 2 + sub) * NT : (half * 2 + sub + 1) * NT],
                                start=True,
                                stop=True,
                            )
                        nc.vector.tensor_tensor(oT[hsl, csl], o_sb[:, csl], rb, ALU.mult)
                    nc.vector.tensor_scalar_add(oT[hsl, :], oT[hsl, :], bv_t[hsl, :])

            # ---- phase 4: out-projection partial (row slice of w_proj) ----
            with (
                tc.tile_pool(name="p4ps", bufs=2, space="PSUM") as p4ps,
                tc.tile_pool(name="p4sb", bufs=2) as p4sb,
            ):
                for j in range(KC):
                    po = p4ps.tile([128, N], F32, tag="po")
                    for nt in range(NTILES):
                        ntsl = slice(nt * NT, (nt + 1) * NT)
                        nc.tensor.matmul(
                            po[:, ntsl],
                            wp_sb[:, j * 128 : (j + 1) * 128],
                            oT[:, ntsl],
                            start=True,
                            stop=True,
                        )
                    osb = p4sb.tile([128, N], F32, tag="osb")
                    nc.vector.tensor_copy(osb, po)
                    nc.sync.dma_start(out=out_part[j * 128 : (j + 1) * 128, :], in_=osb)

    nc.compile()
    return nc


def _host_prep(x, rope, class_mask, w_q, b_q, w_k, w_v, b_v, w_proj):
    """Build per-core input maps. All heavy math stays on device."""
    x2 = np.ascontiguousarray(x.reshape(N, C).astype(np.float32))
    xT = np.ascontiguousarray(x2.T)

    cm = np.asarray(class_mask).reshape(N).astype(bool)
    idx = np.clip(np.cumsum(~cm) - 1, 0, rope.shape[0] - 1)
    sin_m = np.asarray(rope[:, :D], dtype=np.float32)
    cos_m = np.asarray(rope[:, D:], dtype=np.float32)
    sin = np.where(cm[:, None], 0.0, sin_m[idx]).astype(np.float32)  # [N, D]
    cos = np.where(cm[:, None], 1.0, cos_m[idx]).astype(np.float32)
    cosT = np.ascontiguousarray(cos.T)  # [D, N]
    sinT = np.ascontiguousarray(sin.T)
    # sign-fold for the pair-swap trick:
    #   roped[2i]   = q[2i]*cos[2i]   - q[2i+1]*sin[2i]
    #   roped[2i+1] = q[2i+1]*cos[2i+1] + q[2i]*sin[2i+1]
    sgn = np.where(np.arange(D) % 2 == 0, -1.0, 1.0).astype(np.float32)
    sinT_s = sinT * sgn[:, None]
    cos2 = np.concatenate([cosT, cosT], axis=0)  # [128, N] (2 head blocks)
    sin2 = np.concatenate([sinT_s, sinT_s], axis=0)
    scale = np.float32(D ** -0.5)
    cosq = cos2 * scale
    sinq = sin2 * scale
    cosk = cos2
    sink = sin2

    pm = np.zeros((128, 128), dtype=np.float32)
    for i in range(64):
        pm[2 * i + 1, 2 * i] = 1.0
        pm[2 * i, 2 * i + 1] = 1.0
    ident = np.eye(128, dtype=np.float32)
    ones64 = np.ones((1, D), dtype=np.float32)
    vones = np.ones((128, MTILES), dtype=np.float32)

    w_q = np.asarray(w_q, dtype=np.float32)
    w_k = np.asarray(w_k, dtype=np.float32)
    w_v = np.asarray(w_v, dtype=np.float32)
    w_proj = np.asarray(w_proj, dtype=np.float32)
    b_q = np.asarray(b_q, dtype=np.float32)
    b_v = np.asarray(b_v, dtype=np.float32)

    in_maps = []
    for c in range(NCORES):
        csl = slice(c * DH, (c + 1) * DH)
        in_maps.append(
            {
                "xT": xT,
                "wq": np.ascontiguousarray(w_q[:, csl]),
                "wk": np.ascontiguousarray(w_k[:, csl]),
                "wv": np.ascontiguousarray(w_v[:, csl]),
                "wp": np.ascontiguousarray(w_proj[csl, :]),
                "bq": np.ascontiguousarray(b_q[csl].reshape(DH, 1)),
                "bv": np.ascontiguousarray(b_v[csl].reshape(DH, 1)),
                "cosq": cosq,
                "sinq": sinq,
                "cosk": cosk,
                "sink": sink,
                "perm": pm,
                "ident": ident,
                "ones64": ones64,
                "vones": vones,
            }
        )
    return in_maps


def _get_prog():
    global _PROG
    if _PROG is None:
        _PROG = _build_program()
    return _PROG


def kernel(x, rope, class_mask, w_q, b_q, w_k, w_v, b_v, w_proj, b_proj, _trace=False):
    from concourse.bass_utils import run_bass_kernel_spmd

    nc = _get_prog()
    in_maps = _host_prep(x, rope, class_mask, w_q, b_q, w_k, w_v, b_v, w_proj)
    res = run_bass_kernel_spmd(nc, in_maps, core_ids=list(range(NCORES)), trace=_trace)
    acc = np.zeros((C, N), dtype=np.float64)
    for c in range(NCORES):
        acc += res.results[c]["out_part"]
    out = acc.T.astype(np.float32) + np.asarray(b_proj, dtype=np.float32)[None, :]
    out = out.reshape(1, N, C)
    if _trace:
        return out, res
    return out


# revision 5
# speedup vs baseline: 1.1502x; 1.1502x over previous
"""EvaAttention on 8 Trainium2 NeuronCores — head-parallel tensor parallelism.

Per core c (heads 2c, 2c+1):
  - q/k/v projections with column-sliced weights, dh-major layout
    (qT/kT/vT : [128 = 2 heads x 64 dims, 2048 tokens]), fp32r matmuls.
  - Conditional RoPE applied via host-precomputed transposed sin/cos tables
    (identity rows at class-token positions) + a pair-swap permutation matmul.
  - Attention with transposed scores (sT [keys, queries]) so softmax's
    reduction lands on the PE: exp on ScalarE (no max subtraction — scores
    are bounded, fp32), softmax sums via a ones column appended to the V
    stationary operand, normalization after AV. Both heads processed
    together per (query-quarter, key-tile) with row-group-packed QK^T so
    the PE never idles long enough for HAM to re-throttle.
  - Row-sliced out-projection partial product; host sums the 8 partials
    (the tensor-parallel all-reduce, performed at unshard time) + b_proj.
"""

import numpy as np

N = 2048
C = 1024
H = 16
D = 64  # head dim
NCORES = 8
HPC = H // NCORES  # heads per core = 2
DH = HPC * D  # per-core channel slice = 128
NT = 512
NTILES = N // NT  # 4
MT = 128
MTILES = N // MT  # 16
KC = C // 128  # 8 contraction chunks

_PROG = None


def _build_program():
    import concourse.bacc as bacc
    import concourse.mybir as mybir
    import concourse.tile as tile

    F32R = mybir.dt.float32r
    F32 = mybir.dt.float32
    BF16 = mybir.dt.bfloat16
    AF = mybir.ActivationFunctionType
    ALU = mybir.AluOpType

    nc = bacc.Bacc()

    xT = nc.dram_tensor("xT", [C, N], F32R, kind="ExternalInput")
    wq = nc.dram_tensor("wq", [C, DH], F32R, kind="ExternalInput")
    wk = nc.dram_tensor("wk", [C, DH], F32R, kind="ExternalInput")
    wv = nc.dram_tensor("wv", [C, DH], F32R, kind="ExternalInput")
    wp = nc.dram_tensor("wp", [DH, C], F32R, kind="ExternalInput")
    bq = nc.dram_tensor("bq", [DH, 1], F32, kind="ExternalInput")
    bv = nc.dram_tensor("bv", [DH, 1], F32, kind="ExternalInput")
    cosq = nc.dram_tensor("cosq", [DH, N], F32, kind="ExternalInput")
    sinq = nc.dram_tensor("sinq", [DH, N], F32, kind="ExternalInput")
    cosk = nc.dram_tensor("cosk", [DH, N], F32, kind="ExternalInput")
    sink = nc.dram_tensor("sink", [DH, N], F32, kind="ExternalInput")
    perm = nc.dram_tensor("perm", [128, 128], F32R, kind="ExternalInput")
    ident = nc.dram_tensor("ident", [128, 128], F32R, kind="ExternalInput")
    ones64 = nc.dram_tensor("ones64", [1, D], F32R, kind="ExternalInput")
    vones = nc.dram_tensor("vones", [128, MTILES], BF16, kind="ExternalInput")
    out_part = nc.dram_tensor("out_part", [C, N], F32, kind="ExternalOutput")

    with tile.TileContext(nc) as tc, nc.allow_low_precision(reason="fp32r intermediates"):
        with (
            tc.tile_pool(name="const", bufs=1) as const,
            tc.tile_pool(name="work", bufs=1) as work,
        ):
            # ---- small constants first (DMA order matters for the prologue) ----
            wq_sb = const.tile([128, C], F32R)
            wk_sb = const.tile([128, C], F32R)
            wv_sb = const.tile([128, C], F32R)
            for kc in range(KC):
                sl = slice(kc * 128, (kc + 1) * 128)
                nc.sync.dma_start(out=wq_sb[:, sl], in_=wq[sl, :])
                nc.sync.dma_start(out=wk_sb[:, sl], in_=wk[sl, :])
                nc.sync.dma_start(out=wv_sb[:, sl], in_=wv[sl, :])
            bq_t = const.tile([DH, 1], F32)
            bv_t = const.tile([DH, 1], F32)
            nc.sync.dma_start(out=bq_t, in_=bq[:, :])
            nc.sync.dma_start(out=bv_t, in_=bv[:, :])
            perm_t = const.tile([128, 128], F32R)
            ident_t = const.tile([128, 128], F32R)
            nc.sync.dma_start(out=perm_t, in_=perm[:, :])
            nc.sync.dma_start(out=ident_t, in_=ident[:, :])
            ones_t = const.tile([1, D], F32R)
            nc.sync.dma_start(out=ones_t, in_=ones64[:, :])
            vones_t = const.tile([128, MTILES], BF16)
            nc.sync.dma_start(out=vones_t, in_=vones[:, :])
            wp_sb = const.tile([128, C], F32R)
            nc.sync.dma_start(out=wp_sb, in_=wp[:, :])

            # ---- x^T streamed per (chunk, n-tile) so projections start early ----
            xts = [const.tile([128, N], F32R, name=f"xts{kc}", tag=f"xts{kc}") for kc in range(KC)]
            for nt in range(NTILES):
                ntsl = slice(nt * NT, (nt + 1) * NT)
                for kc in range(KC):
                    nc.sync.dma_start(out=xts[kc][:, ntsl], in_=xT[kc * 128 : (kc + 1) * 128, ntsl])

            # ---- persistent work tiles ----
            qT = work.tile([DH, N], F32R)  # roped, pre-scaled by d^-0.5
            kT = work.tile([DH, N], F32R)  # roped
            vT = work.tile([DH, N], F32R)
            va = [work.tile([128, MTILES * (D + 1)], BF16, name=f"va{h}", tag=f"va{h}") for h in range(HPC)]
            va3 = [v.rearrange("p (t e) -> p t e", e=D + 1) for v in va]
            oT = work.tile([DH, N], F32R)  # normalized per-head outputs (+ b_v)

            # ---- phase 1: projections + rope ----
            with (
                tc.tile_pool(name="p1sb", bufs=2) as p1sb,
                tc.tile_pool(name="p1ps", bufs=3, space="PSUM") as p1ps,
                tc.tile_pool(name="p1sw", bufs=2, space="PSUM") as p1sw,
            ):
                for nt in range(NTILES):
                    ntsl = slice(nt * NT, (nt + 1) * NT)
                    # v projection (no rope)
                    psv = p1ps.tile([128, NT], F32, tag="proj")
                    for kc in range(KC):
                        nc.tensor.matmul(
                            psv,
                            wv_sb[:, kc * 128 : (kc + 1) * 128],
                            xts[kc][:, ntsl],
                            start=(kc == 0),
                            stop=(kc == KC - 1),
                        )
                    nc.scalar.copy(vT[:, ntsl], psv)
                    # q and k projections + rope
                    for which in ("q", "k"):
                        ps = p1ps.tile([128, NT], F32, tag="proj", name=f"ps_{which}{nt}")
                        w_sb = wq_sb if which == "q" else wk_sb
                        for kc in range(KC):
                            nc.tensor.matmul(
                                ps,
                                w_sb[:, kc * 128 : (kc + 1) * 128],
                                xts[kc][:, ntsl],
                                start=(kc == 0),
                                stop=(kc == KC - 1),
                            )
                        raw = p1sb.tile([128, NT], F32R, tag="raw", name=f"raw_{which}{nt}")
                        if which == "q":
                            nc.scalar.activation(raw, ps, AF.Identity, bias=bq_t)
                        else:
                            nc.scalar.copy(raw, ps)
                        # pair-swap via permutation matmul
                        psw = p1sw.tile([128, NT], F32, tag="swap", name=f"psw_{which}{nt}")
                        nc.tensor.matmul(psw, perm_t, raw, start=True, stop=True)
                        cos_d = cosq if which == "q" else cosk
                        sin_d = sinq if which == "q" else sink
                        cs = p1sb.tile([128, NT], F32, tag="cs", name=f"cs_{which}{nt}")
                        sn = p1sb.tile([128, NT], F32, tag="sn", name=f"sn_{which}{nt}")
                        nc.sync.dma_start(out=cs, in_=cos_d[:, ntsl])
                        nc.sync.dma_start(out=sn, in_=sin_d[:, ntsl])
                        t1 = p1sb.tile([128, NT], F32, tag="t1", name=f"t1_{which}{nt}")
                        nc.vector.tensor_tensor(t1, psw, sn, ALU.mult)
                        dst = qT if which == "q" else kT
                        nc.vector.tensor_tensor(dst[:, ntsl], raw, cs, ALU.mult)
                        nc.vector.tensor_tensor(dst[:, ntsl], dst[:, ntsl], t1, ALU.add)

            # ---- phase 2: transpose v to token-major, append ones column ----
            for h in range(HPC):
                nc.sync.dma_start(out=va3[h][:, :, D], in_=vones_t[:, :])
            with tc.tile_pool(name="p2ps", bufs=2, space="PSUM") as p2ps:
                for mt in range(MTILES):
                    pst = p2ps.tile([128, 128], F32R, tag="tr")
                    nc.tensor.transpose(pst, vT[:, mt * 128 : (mt + 1) * 128], ident_t)
                    for h in range(HPC):
                        nc.vector.tensor_copy(va3[h][:, mt, 0:D], pst[:, h * D : (h + 1) * D])

            # ---- phase 3: attention, both heads interleaved per query-quarter ----
            with (
                tc.tile_pool(name="p3o", bufs=1, space="PSUM") as p3o,
                tc.tile_pool(name="p3s", bufs=3, space="PSUM") as p3s,
                tc.tile_pool(name="p3sb", bufs=3) as p3sb,
                tc.tile_pool(name="p3misc", bufs=2) as p3misc,
            ):
                for nq in range(NTILES):
                    nqsl = slice(nq * NT, (nq + 1) * NT)
                    o_ps = [
                        p3o.tile([D + 1, NT], F32, tag=f"o{h}", name=f"o{h}_{nq}")
                        for h in range(HPC)
                    ]
                    for mt in range(MTILES):
                        mtsl = slice(mt * 128, (mt + 1) * 128)
                        ps_s = p3s.tile([128, 2 * NT], F32, tag="s", name=f"s{nq}_{mt}")
                        for h in range(HPC):
                            hsl = slice(h * D, (h + 1) * D)
                            nc.tensor.matmul(
                                ps_s[:, h * NT : (h + 1) * NT],
                                kT[hsl, mtsl],
                                qT[hsl, nqsl],
                                start=True,
                                stop=True,
                            )
                        pT = p3sb.tile([128, 2 * NT], BF16, tag="p", name=f"p{nq}_{mt}")
                        nc.scalar.activation(pT, ps_s, AF.Exp)
                        for h in range(HPC):
                            nc.tensor.matmul(
                                o_ps[h],
                                va3[h][:, mt, :],
                                pT[:, h * NT : (h + 1) * NT],
                                start=(mt == 0),
                                stop=(mt == MTILES - 1),
                            )
                    # per-quarter epilogue: normalize rows 0..63 by row 64 (softmax sum)
                    for h in range(HPC):
                        hsl = slice(h * D, (h + 1) * D)
                        sums = p3misc.tile([1, NT], F32R, tag="sums", name=f"sm{h}_{nq}")
                        nc.scalar.copy(sums, o_ps[h][D : D + 1, :])
                        rb = p3s.tile([D, NT], F32, tag="s", name=f"rb{h}_{nq}")
                        nc.tensor.matmul(rb, ones_t, sums, start=True, stop=True)
                        rs = p3misc.tile([D, NT], F32, tag="rs", name=f"rs{h}_{nq}")
                        nc.vector.reciprocal(rs, rb)
                        nc.vector.tensor_tensor(oT[hsl, nqsl], o_ps[h][0:D, :], rs, ALU.mult)
                nc.vector.tensor_scalar_add(oT, oT, bv_t)

            # ---- phase 4: out-projection partial (row slice of w_proj) ----
            with (
                tc.tile_pool(name="p4ps", bufs=2, space="PSUM") as p4ps,
                tc.tile_pool(name="p4sb", bufs=2) as p4sb,
            ):
                for j in range(KC):
                    po = p4ps.tile([128, N], F32, tag="po")
                    for nt in range(NTILES):
                        ntsl = slice(nt * NT, (nt + 1) * NT)
                        nc.tensor.matmul(
                            po[:, ntsl],
                            wp_sb[:, j * 128 : (j + 1) * 128],
                            oT[:, ntsl],
                            start=True,
                            stop=True,
                        )
                    osb = p4sb.tile([128, N], F32, tag="osb")
                    if j % 2 == 0:
                        nc.vector.tensor_copy(osb, po)
                    else:
                        nc.scalar.copy(osb, po)
                    nc.sync.dma_start(out=out_part[j * 128 : (j + 1) * 128, :], in_=osb)

    nc.compile()
    return nc


def _host_prep(x, rope, class_mask, w_q, b_q, w_k, w_v, b_v, w_proj):
    """Build per-core input maps. All heavy math stays on device."""
    import ml_dtypes

    x2 = np.ascontiguousarray(x.reshape(N, C).astype(np.float32))
    xT = np.ascontiguousarray(x2.T)

    cm = np.asarray(class_mask).reshape(N).astype(bool)
    idx = np.clip(np.cumsum(~cm) - 1, 0, rope.shape[0] - 1)
    sin_m = np.asarray(rope[:, :D], dtype=np.float32)
    cos_m = np.asarray(rope[:, D:], dtype=np.float32)
    sin = np.where(cm[:, None], 0.0, sin_m[idx]).astype(np.float32)  # [N, D]
    cos = np.where(cm[:, None], 1.0, cos_m[idx]).astype(np.float32)
    cosT = np.ascontiguousarray(cos.T)  # [D, N]
    sinT = np.ascontiguousarray(sin.T)
    # sign-fold for the pair-swap trick:
    #   roped[2i]   = q[2i]*cos[2i]   - q[2i+1]*sin[2i]
    #   roped[2i+1] = q[2i+1]*cos[2i+1] + q[2i]*sin[2i+1]
    sgn = np.where(np.arange(D) % 2 == 0, -1.0, 1.0).astype(np.float32)
    sinT_s = sinT * sgn[:, None]
    cos2 = np.concatenate([cosT, cosT], axis=0)  # [128, N] (2 head blocks)
    sin2 = np.concatenate([sinT_s, sinT_s], axis=0)
    scale = np.float32(D ** -0.5)
    cosq = cos2 * scale
    sinq = sin2 * scale
    cosk = cos2
    sink = sin2

    pm = np.zeros((128, 128), dtype=np.float32)
    for i in range(64):
        pm[2 * i + 1, 2 * i] = 1.0
        pm[2 * i, 2 * i + 1] = 1.0
    ident = np.eye(128, dtype=np.float32)
    ones64 = np.ones((1, D), dtype=np.float32)
    vones = np.ones((128, MTILES), dtype=ml_dtypes.bfloat16)

    w_q = np.asarray(w_q, dtype=np.float32)
    w_k = np.asarray(w_k, dtype=np.float32)
    w_v = np.asarray(w_v, dtype=np.float32)
    w_proj = np.asarray(w_proj, dtype=np.float32)
    b_q = np.asarray(b_q, dtype=np.float32)
    b_v = np.asarray(b_v, dtype=np.float32)

    in_maps = []
    for c in range(NCORES):
        csl = slice(c * DH, (c + 1) * DH)
        in_maps.append(
            {
                "xT": xT,
                "wq": np.ascontiguousarray(w_q[:, csl]),
                "wk": np.ascontiguousarray(w_k[:, csl]),
                "wv": np.ascontiguousarray(w_v[:, csl]),
                "wp": np.ascontiguousarray(w_proj[csl, :]),
                "bq": np.ascontiguousarray(b_q[csl].reshape(DH, 1)),
                "bv": np.ascontiguousarray(b_v[csl].reshape(DH, 1)),
                "cosq": cosq,
                "sinq": sinq,
                "cosk": cosk,
                "sink": sink,
                "perm": pm,
                "ident": ident,
                "ones64": ones64,
                "vones": vones,
            }
        )
    return in_maps


def _get_prog():
    global _PROG
    if _PROG is None:
        _PROG = _build_program()
    return _PROG


def kernel(x, rope, class_mask, w_q, b_q, w_k, w_v, b_v, w_proj, b_proj, _trace=False):
    from concourse.bass_utils import run_bass_kernel_spmd

    nc = _get_prog()
    in_maps = _host_prep(x, rope, class_mask, w_q, b_q, w_k, w_v, b_v, w_proj)
    res = run_bass_kernel_spmd(nc, in_maps, core_ids=list(range(NCORES)), trace=_trace)
    acc = np.zeros((C, N), dtype=np.float64)
    for c in range(NCORES):
        acc += res.results[c]["out_part"]
    out = acc.T.astype(np.float32) + np.asarray(b_proj, dtype=np.float32)[None, :]
    out = out.reshape(1, N, C)
    if _trace:
        return out, res
    return out


# revision 6
# speedup vs baseline: 1.1971x; 1.0407x over previous
"""EvaAttention on 8 Trainium2 NeuronCores — head-parallel tensor parallelism.

Per core c (heads 2c, 2c+1):
  - q/k/v projections with column-sliced weights, dh-major layout
    (qT/kT/vT : [128 = 2 heads x 64 dims, 2048 tokens]), bf16 matmuls with
    fp32 PSUM accumulation (bf16 sustains ~131ns/MM vs ~600ns for fp32r
    self-loading matmuls on TRN2).
  - Conditional RoPE applied via host-precomputed transposed sin/cos tables
    (identity rows at class-token positions) + a pair-swap permutation matmul.
  - Attention with transposed scores (sT [keys, queries]) so softmax's
    reduction lands on the PE: exp on ScalarE (no max subtraction — scores
    are bounded, fp32 psum), softmax sums via a ones column appended to the
    V stationary operand, normalization after AV with
    reciprocal_approx_fast. Both heads processed together per
    (query-quarter, key-tile) with row-group-packed QK^T so the PE never
    idles long enough for HAM to re-throttle.
  - Row-sliced out-projection partial interleaved per query-quarter (hides
    the output DMA under the next quarter's attention); host sums the 8
    partials (the tensor-parallel all-reduce at unshard time) + b_proj.
"""

import numpy as np

N = 2048
C = 1024
H = 16
D = 64  # head dim
NCORES = 8
HPC = H // NCORES  # heads per core = 2
DH = HPC * D  # per-core channel slice = 128
NT = 512
NTILES = N // NT  # 4
MT = 128
MTILES = N // MT  # 16
KC = C // 128  # 8 contraction chunks

_PROG = None


def _build_program():
    import concourse.bacc as bacc
    import concourse.mybir as mybir
    import concourse.tile as tile

    F32R = mybir.dt.float32r
    F32 = mybir.dt.float32
    BF16 = mybir.dt.bfloat16
    AF = mybir.ActivationFunctionType
    ALU = mybir.AluOpType

    nc = bacc.Bacc()

    xT = nc.dram_tensor("xT", [C, N], BF16, kind="ExternalInput")
    wq = nc.dram_tensor("wq", [C, DH], BF16, kind="ExternalInput")
    wk = nc.dram_tensor("wk", [C, DH], BF16, kind="ExternalInput")
    wv = nc.dram_tensor("wv", [C, DH], BF16, kind="ExternalInput")
    wp = nc.dram_tensor("wp", [DH, C], BF16, kind="ExternalInput")
    bq = nc.dram_tensor("bq", [DH, 1], F32, kind="ExternalInput")
    bv = nc.dram_tensor("bv", [DH, 1], F32, kind="ExternalInput")
    cosq = nc.dram_tensor("cosq", [DH, N], F32, kind="ExternalInput")
    sinq = nc.dram_tensor("sinq", [DH, N], F32, kind="ExternalInput")
    cosk = nc.dram_tensor("cosk", [DH, N], F32, kind="ExternalInput")
    sink = nc.dram_tensor("sink", [DH, N], F32, kind="ExternalInput")
    perm = nc.dram_tensor("perm", [128, 128], BF16, kind="ExternalInput")
    ident = nc.dram_tensor("ident", [128, 128], BF16, kind="ExternalInput")
    ones64 = nc.dram_tensor("ones64", [1, D], F32R, kind="ExternalInput")
    vones = nc.dram_tensor("vones", [128, MTILES], BF16, kind="ExternalInput")
    out_part = nc.dram_tensor("out_part", [C, N], F32, kind="ExternalOutput")

    with tile.TileContext(nc) as tc, nc.allow_low_precision(reason="bf16/fp32r intermediates"):
        with (
            tc.tile_pool(name="const", bufs=1) as const,
            tc.tile_pool(name="work", bufs=1) as work,
        ):
            # ---- small constants first (DMA order matters for the prologue) ----
            wq_sb = const.tile([128, C], BF16)
            wk_sb = const.tile([128, C], BF16)
            wv_sb = const.tile([128, C], BF16)
            for kc in range(KC):
                sl = slice(kc * 128, (kc + 1) * 128)
                nc.sync.dma_start(out=wq_sb[:, sl], in_=wq[sl, :])
                nc.sync.dma_start(out=wk_sb[:, sl], in_=wk[sl, :])
                nc.sync.dma_start(out=wv_sb[:, sl], in_=wv[sl, :])
            bq_t = const.tile([DH, 1], F32)
            bv_t = const.tile([DH, 1], F32)
            nc.sync.dma_start(out=bq_t, in_=bq[:, :])
            nc.sync.dma_start(out=bv_t, in_=bv[:, :])
            perm_t = const.tile([128, 128], BF16)
            ident_t = const.tile([128, 128], BF16)
            nc.sync.dma_start(out=perm_t, in_=perm[:, :])
            nc.sync.dma_start(out=ident_t, in_=ident[:, :])
            ones_t = const.tile([1, D], F32R)
            nc.sync.dma_start(out=ones_t, in_=ones64[:, :])
            vones_t = const.tile([128, MTILES], BF16)
            nc.sync.dma_start(out=vones_t, in_=vones[:, :])
            wp_sb = const.tile([128, C], BF16)
            nc.sync.dma_start(out=wp_sb, in_=wp[:, :])

            # ---- x^T streamed per (chunk, n-tile) so projections start early ----
            xts = [const.tile([128, N], BF16, name=f"xts{kc}", tag=f"xts{kc}") for kc in range(KC)]
            for nt in range(NTILES):
                ntsl = slice(nt * NT, (nt + 1) * NT)
                for kc in range(KC):
                    nc.sync.dma_start(out=xts[kc][:, ntsl], in_=xT[kc * 128 : (kc + 1) * 128, ntsl])

            # ---- persistent work tiles ----
            qT = work.tile([DH, N], BF16)  # roped, pre-scaled by d^-0.5
            kT = work.tile([DH, N], BF16)  # roped
            vT = work.tile([DH, N], BF16)
            va = [work.tile([128, MTILES * (D + 1)], BF16, name=f"va{h}", tag=f"va{h}") for h in range(HPC)]
            va3 = [v.rearrange("p (t e) -> p t e", e=D + 1) for v in va]
            oT = work.tile([DH, N], BF16)  # normalized per-head outputs (+ b_v)

            # ---- phase 1: projections + rope ----
            with (
                tc.tile_pool(name="p1sb", bufs=2) as p1sb,
                tc.tile_pool(name="p1ps", bufs=3, space="PSUM") as p1ps,
                tc.tile_pool(name="p1sw", bufs=2, space="PSUM") as p1sw,
            ):
                for nt in range(NTILES):
                    ntsl = slice(nt * NT, (nt + 1) * NT)
                    # v projection (no rope)
                    psv = p1ps.tile([128, NT], F32, tag="proj")
                    for kc in range(KC):
                        nc.tensor.matmul(
                            psv,
                            wv_sb[:, kc * 128 : (kc + 1) * 128],
                            xts[kc][:, ntsl],
                            start=(kc == 0),
                            stop=(kc == KC - 1),
                        )
                    nc.scalar.copy(vT[:, ntsl], psv)
                    # q and k projections + rope
                    for which in ("q", "k"):
                        ps = p1ps.tile([128, NT], F32, tag="proj", name=f"ps_{which}{nt}")
                        w_sb = wq_sb if which == "q" else wk_sb
                        for kc in range(KC):
                            nc.tensor.matmul(
                                ps,
                                w_sb[:, kc * 128 : (kc + 1) * 128],
                                xts[kc][:, ntsl],
                                start=(kc == 0),
                                stop=(kc == KC - 1),
                            )
                        raw = p1sb.tile([128, NT], BF16, tag="raw", name=f"raw_{which}{nt}")
                        if which == "q":
                            nc.scalar.activation(raw, ps, AF.Identity, bias=bq_t)
                        else:
                            nc.scalar.copy(raw, ps)
                        # pair-swap via permutation matmul
                        psw = p1sw.tile([128, NT], F32, tag="swap", name=f"psw_{which}{nt}")
                        nc.tensor.matmul(psw, perm_t, raw, start=True, stop=True)
                        cos_d = cosq if which == "q" else cosk
                        sin_d = sinq if which == "q" else sink
                        cs = p1sb.tile([128, NT], F32, tag="cs", name=f"cs_{which}{nt}")
                        sn = p1sb.tile([128, NT], F32, tag="sn", name=f"sn_{which}{nt}")
                        nc.sync.dma_start(out=cs, in_=cos_d[:, ntsl])
                        nc.sync.dma_start(out=sn, in_=sin_d[:, ntsl])
                        t1 = p1sb.tile([128, NT], F32, tag="t1", name=f"t1_{which}{nt}")
                        nc.vector.tensor_tensor(t1, psw, sn, ALU.mult)
                        dst = qT if which == "q" else kT
                        nc.vector.tensor_tensor(dst[:, ntsl], raw, cs, ALU.mult)
                        nc.vector.tensor_tensor(dst[:, ntsl], dst[:, ntsl], t1, ALU.add)

            # ---- phase 2: transpose v to token-major, append ones column ----
            for h in range(HPC):
                nc.sync.dma_start(out=va3[h][:, :, D], in_=vones_t[:, :])
            with tc.tile_pool(name="p2ps", bufs=2, space="PSUM") as p2ps:
                for mt in range(MTILES):
                    pst = p2ps.tile([128, 128], BF16, tag="tr")
                    nc.tensor.transpose(pst, vT[:, mt * 128 : (mt + 1) * 128], ident_t)
                    for h in range(HPC):
                        nc.vector.tensor_copy(va3[h][:, mt, 0:D], pst[:, h * D : (h + 1) * D])

            # ---- phase 3: attention, both heads interleaved per query-quarter,
            # ----          out-projection chunk interleaved per quarter
            with (
                tc.tile_pool(name="p3o", bufs=1, space="PSUM") as p3o,
                tc.tile_pool(name="p3s", bufs=3, space="PSUM") as p3s,
                tc.tile_pool(name="p3sb", bufs=4) as p3sb,
                tc.tile_pool(name="p3misc", bufs=2) as p3misc,
                tc.tile_pool(name="p3out", bufs=2) as p3out,
            ):
                for nq in range(NTILES):
                    nqsl = slice(nq * NT, (nq + 1) * NT)
                    o_ps = [
                        p3o.tile([D + 1, NT], F32, tag=f"o{h}", name=f"o{h}_{nq}")
                        for h in range(HPC)
                    ]
                    for mt in range(MTILES):
                        mtsl = slice(mt * 128, (mt + 1) * 128)
                        ps_s = p3s.tile([128, 2 * NT], F32, tag="s", name=f"s{nq}_{mt}")
                        for h in range(HPC):
                            hsl = slice(h * D, (h + 1) * D)
                            nc.tensor.matmul(
                                ps_s[:, h * NT : (h + 1) * NT],
                                kT[hsl, mtsl],
                                qT[hsl, nqsl],
                                start=True,
                                stop=True,
                            )
                        pT = p3sb.tile([128, 2 * NT], BF16, tag="p", name=f"p{nq}_{mt}")
                        nc.scalar.activation(pT, ps_s, AF.Exp)
                        for h in range(HPC):
                            nc.tensor.matmul(
                                o_ps[h],
                                va3[h][:, mt, :],
                                pT[:, h * NT : (h + 1) * NT],
                                start=(mt == 0),
                                stop=(mt == MTILES - 1),
                            )
                    # per-quarter epilogue: normalize rows 0..63 by row 64 (softmax sum)
                    for h in range(HPC):
                        hsl = slice(h * D, (h + 1) * D)
                        sums = p3misc.tile([1, NT], F32R, tag="sums", name=f"sm{h}_{nq}")
                        nc.scalar.copy(sums, o_ps[h][D : D + 1, :])
                        rb = p3s.tile([D, NT], F32, tag="s", name=f"rb{h}_{nq}")
                        nc.tensor.matmul(rb, ones_t, sums, start=True, stop=True)
                        rs = p3misc.tile([D, NT], F32, tag="rs", name=f"rs{h}_{nq}")
                        nc.vector.reciprocal_approx_fast(rs, rb)
                        nc.vector.tensor_tensor(oT[hsl, nqsl], o_ps[h][0:D, :], rs, ALU.mult)
                        nc.vector.tensor_scalar_add(oT[hsl, nqsl], oT[hsl, nqsl], bv_t[hsl, :])
                    # out-projection for this quarter (overlaps next quarter's attention)
                    for j in range(KC):
                        po = p3s.tile([128, NT], F32, tag="s", name=f"po{j}_{nq}")
                        nc.tensor.matmul(
                            po,
                            wp_sb[:, j * 128 : (j + 1) * 128],
                            oT[:, nqsl],
                            start=True,
                            stop=True,
                        )
                        osb = p3out.tile([128, NT], F32, tag="osb", name=f"osb{j}_{nq}")
                        nc.vector.tensor_copy(osb, po)
                        nc.sync.dma_start(out=out_part[j * 128 : (j + 1) * 128, nqsl], in_=osb)

    nc.compile()
    return nc


def _host_prep(x, rope, class_mask, w_q, b_q, w_k, w_v, b_v, w_proj):
    """Build per-core input maps. All heavy math stays on device."""
    import ml_dtypes

    BF = ml_dtypes.bfloat16
    x2 = np.ascontiguousarray(x.reshape(N, C).astype(np.float32))
    xT = np.ascontiguousarray(x2.T.astype(BF))

    cm = np.asarray(class_mask).reshape(N).astype(bool)
    idx = np.clip(np.cumsum(~cm) - 1, 0, rope.shape[0] - 1)
    sin_m = np.asarray(rope[:, :D], dtype=np.float32)
    cos_m = np.asarray(rope[:, D:], dtype=np.float32)
    sin = np.where(cm[:, None], 0.0, sin_m[idx]).astype(np.float32)  # [N, D]
    cos = np.where(cm[:, None], 1.0, cos_m[idx]).astype(np.float32)
    cosT = np.ascontiguousarray(cos.T)  # [D, N]
    sinT = np.ascontiguousarray(sin.T)
    # sign-fold for the pair-swap trick:
    #   roped[2i]   = q[2i]*cos[2i]   - q[2i+1]*sin[2i]
    #   roped[2i+1] = q[2i+1]*cos[2i+1] + q[2i]*sin[2i+1]
    sgn = np.where(np.arange(D) % 2 == 0, -1.0, 1.0).astype(np.float32)
    sinT_s = sinT * sgn[:, None]
    cos2 = np.concatenate([cosT, cosT], axis=0)  # [128, N] (2 head blocks)
    sin2 = np.concatenate([sinT_s, sinT_s], axis=0)
    scale = np.float32(D ** -0.5)
    cosq = cos2 * scale
    sinq = sin2 * scale
    cosk = cos2
    sink = sin2

    pm = np.zeros((128, 128), dtype=np.float32)
    for i in range(64):
        pm[2 * i + 1, 2 * i] = 1.0
        pm[2 * i, 2 * i + 1] = 1.0
    ident = np.eye(128, dtype=BF)
    ones64 = np.ones((1, D), dtype=np.float32)
    vones = np.ones((128, MTILES), dtype=BF)

    w_q = np.asarray(w_q, dtype=np.float32)
    w_k = np.asarray(w_k, dtype=np.float32)
    w_v = np.asarray(w_v, dtype=np.float32)
    w_proj = np.asarray(w_proj, dtype=np.float32)
    b_q = np.asarray(b_q, dtype=np.float32)
    b_v = np.asarray(b_v, dtype=np.float32)

    in_maps = []
    for c in range(NCORES):
        csl = slice(c * DH, (c + 1) * DH)
        in_maps.append(
            {
                "xT": xT,
                "wq": np.ascontiguousarray(w_q[:, csl].astype(BF)),
                "wk": np.ascontiguousarray(w_k[:, csl].astype(BF)),
                "wv": np.ascontiguousarray(w_v[:, csl].astype(BF)),
                "wp": np.ascontiguousarray(w_proj[csl, :].astype(BF)),
                "bq": np.ascontiguousarray(b_q[csl].reshape(DH, 1)),
                "bv": np.ascontiguousarray(b_v[csl].reshape(DH, 1)),
                "cosq": cosq,
                "sinq": sinq,
                "cosk": cosk,
                "sink": sink,
                "perm": pm.astype(BF),
                "ident": ident,
                "ones64": ones64,
                "vones": vones,
            }
        )
    return in_maps


def _get_prog():
    global _PROG
    if _PROG is None:
        _PROG = _build_program()
    return _PROG


def kernel(x, rope, class_mask, w_q, b_q, w_k, w_v, b_v, w_proj, b_proj, _trace=False):
    from concourse.bass_utils import run_bass_kernel_spmd

    nc = _get_prog()
    in_maps = _host_prep(x, rope, class_mask, w_q, b_q, w_k, w_v, b_v, w_proj)
    res = run_bass_kernel_spmd(nc, in_maps, core_ids=list(range(NCORES)), trace=_trace)
    acc = np.zeros((C, N), dtype=np.float64)
    for c in range(NCORES):
        acc += res.results[c]["out_part"]
    out = acc.T.astype(np.float32) + np.asarray(b_proj, dtype=np.float32)[None, :]
    out = out.reshape(1, N, C)
    if _trace:
        return out, res
    return out


# revision 8
# speedup vs baseline: 1.2777x; 1.0674x over previous
"""EvaAttention on 8 Trainium2 NeuronCores — head-parallel tensor parallelism.

Per core c (heads 2c, 2c+1):
  - q/k/v projections with column-sliced weights, dh-major layout
    (qT/kT/vT : [128 = 2 heads x 64 dims, 2048 tokens]), fp16 matmuls with
    fp32 PSUM accumulation (fp16 streams 1 cycle/row like bf16 — ~131ns per
    N=512 matmul vs ~600ns for fp32r self-loading — at 4x finer mantissa).
  - Conditional RoPE applied via host-precomputed transposed sin/cos tables
    (identity rows at class-token positions) + a pair-swap permutation matmul.
  - Attention with transposed scores (sT [keys, queries]) so softmax's
    reduction lands on the PE: exp(s - 12) on ScalarE (constant shift keeps
    exp within fp16 range; softmax is shift-invariant), softmax sums via a
    ones column appended to the V stationary operand, normalization after AV
    with reciprocal_approx_fast. Both heads processed together per
    (query-quarter, key-tile) with row-group-packed QK^T so the PE never
    idles long enough for HAM to re-throttle.
  - Row-sliced out-projection partial interleaved per query-quarter (hides
    the output DMA under the next quarter's attention); host sums the 8
    partials (the tensor-parallel all-reduce at unshard time) + b_proj.
"""

import numpy as np

N = 2048
C = 1024
H = 16
D = 64  # head dim
NCORES = 8
HPC = H // NCORES  # heads per core = 2
DH = HPC * D  # per-core channel slice = 128
NT = 512
NTILES = N // NT  # 4
MT = 128
MTILES = N // MT  # 16
KC = C // 128  # 8 contraction chunks
ESHIFT = -12.0  # exp(s + ESHIFT): keeps exp in fp16 range; cancels in softmax

_PROG = None


def _build_program():
    import concourse.bacc as bacc
    import concourse.mybir as mybir
    import concourse.tile as tile

    F32R = mybir.dt.float32r
    F32 = mybir.dt.float32
    F16 = mybir.dt.float16
    AF = mybir.ActivationFunctionType
    ALU = mybir.AluOpType

    nc = bacc.Bacc()

    xT = nc.dram_tensor("xT", [C, N], F16, kind="ExternalInput")
    wq = nc.dram_tensor("wq", [C, DH], F16, kind="ExternalInput")
    wk = nc.dram_tensor("wk", [C, DH], F16, kind="ExternalInput")
    wv = nc.dram_tensor("wv", [C, DH], F16, kind="ExternalInput")
    wp = nc.dram_tensor("wp", [DH, C], F16, kind="ExternalInput")
    bq = nc.dram_tensor("bq", [DH, 1], F32, kind="ExternalInput")
    bv = nc.dram_tensor("bv", [DH, 1], F32, kind="ExternalInput")
    cosq = nc.dram_tensor("cosq", [DH, N], F16, kind="ExternalInput")
    sinq = nc.dram_tensor("sinq", [DH, N], F16, kind="ExternalInput")
    cosk = nc.dram_tensor("cosk", [DH, N], F16, kind="ExternalInput")
    sink = nc.dram_tensor("sink", [DH, N], F16, kind="ExternalInput")
    perm = nc.dram_tensor("perm", [128, 128], F16, kind="ExternalInput")
    ident = nc.dram_tensor("ident", [128, 128], F16, kind="ExternalInput")
    ones64 = nc.dram_tensor("ones64", [1, D], F32R, kind="ExternalInput")
    vones = nc.dram_tensor("vones", [128, MTILES], F16, kind="ExternalInput")
    out_part = nc.dram_tensor("out_part", [C, N], F32, kind="ExternalOutput")

    with tile.TileContext(nc) as tc, nc.allow_low_precision(reason="fp16 intermediates, fp32 accumulate"):
        with (
            tc.tile_pool(name="const", bufs=1) as const,
            tc.tile_pool(name="work", bufs=1) as work,
        ):
            # ---- small constants first (single 3D-AP DMAs keep the issue queue short) ----
            wq_sb = const.tile([128, C], F16)
            wk_sb = const.tile([128, C], F16)
            wv_sb = const.tile([128, C], F16)
            for w_sb, w_d in ((wq_sb, wq), (wk_sb, wk), (wv_sb, wv)):
                nc.sync.dma_start(
                    out=w_sb.rearrange("p (kc d) -> p kc d", d=DH),
                    in_=w_d.rearrange("(kc p) d -> p kc d", p=128),
                )
            bq_t = const.tile([DH, 1], F32)
            bv_t = const.tile([DH, 1], F32)
            nc.sync.dma_start(out=bq_t, in_=bq[:, :])
            nc.sync.dma_start(out=bv_t, in_=bv[:, :])
            perm_t = const.tile([128, 128], F16)
            ident_t = const.tile([128, 128], F16)
            nc.sync.dma_start(out=perm_t, in_=perm[:, :])
            nc.sync.dma_start(out=ident_t, in_=ident[:, :])
            ones_t = const.tile([1, D], F32R)
            nc.sync.dma_start(out=ones_t, in_=ones64[:, :])
            vones_t = const.tile([128, MTILES], F16)
            nc.sync.dma_start(out=vones_t, in_=vones[:, :])
            wp_sb = const.tile([128, C], F16)
            nc.sync.dma_start(out=wp_sb, in_=wp[:, :])
            eshift_t = const.tile([128, 1], F32)
            nc.vector.memset(eshift_t, ESHIFT)

            # ---- x^T: one DMA per 128-channel chunk ----
            xts = [const.tile([128, N], F16, name=f"xts{kc}", tag=f"xts{kc}") for kc in range(KC)]
            for kc in range(KC):
                nc.sync.dma_start(out=xts[kc], in_=xT[kc * 128 : (kc + 1) * 128, :])

            # ---- persistent work tiles ----
            qT = work.tile([DH, N], F16)  # roped, pre-scaled by d^-0.5
            kT = work.tile([DH, N], F16)  # roped
            vT = work.tile([DH, N], F16)
            va = [work.tile([128, MTILES * (D + 1)], F16, name=f"va{h}", tag=f"va{h}") for h in range(HPC)]
            va3 = [v.rearrange("p (t e) -> p t e", e=D + 1) for v in va]
            oT = work.tile([DH, N], F16)  # normalized per-head outputs (+ b_v)

            # ---- phase 1: projections + rope ----
            with (
                tc.tile_pool(name="p1sb", bufs=2) as p1sb,
                tc.tile_pool(name="p1ps", bufs=3, space="PSUM") as p1ps,
                tc.tile_pool(name="p1sw", bufs=2, space="PSUM") as p1sw,
            ):
                for nt in range(NTILES):
                    ntsl = slice(nt * NT, (nt + 1) * NT)
                    # v projection (no rope)
                    psv = p1ps.tile([128, NT], F32, tag="proj")
                    for kc in range(KC):
                        nc.tensor.matmul(
                            psv,
                            wv_sb[:, kc * 128 : (kc + 1) * 128],
                            xts[kc][:, ntsl],
                            start=(kc == 0),
                            stop=(kc == KC - 1),
                        )
                    nc.scalar.copy(vT[:, ntsl], psv)
                    # q and k projections + rope
                    for which in ("q", "k"):
                        ps = p1ps.tile([128, NT], F32, tag="proj", name=f"ps_{which}{nt}")
                        w_sb = wq_sb if which == "q" else wk_sb
                        for kc in range(KC):
                            nc.tensor.matmul(
                                ps,
                                w_sb[:, kc * 128 : (kc + 1) * 128],
                                xts[kc][:, ntsl],
                                start=(kc == 0),
                                stop=(kc == KC - 1),
                            )
                        raw = p1sb.tile([128, NT], F16, tag="raw", name=f"raw_{which}{nt}")
                        if which == "q":
                            nc.scalar.activation(raw, ps, AF.Identity, bias=bq_t)
                        else:
                            nc.scalar.copy(raw, ps)
                        # pair-swap via permutation matmul
                        psw = p1sw.tile([128, NT], F32, tag="swap", name=f"psw_{which}{nt}")
                        nc.tensor.matmul(psw, perm_t, raw, start=True, stop=True)
                        cos_d = cosq if which == "q" else cosk
                        sin_d = sinq if which == "q" else sink
                        cs = p1sb.tile([128, NT], F16, tag="cs", name=f"cs_{which}{nt}")
                        sn = p1sb.tile([128, NT], F16, tag="sn", name=f"sn_{which}{nt}")
                        nc.gpsimd.dma_start(out=cs, in_=cos_d[:, ntsl])
                        nc.gpsimd.dma_start(out=sn, in_=sin_d[:, ntsl])
                        t1 = p1sb.tile([128, NT], F32, tag="t1", name=f"t1_{which}{nt}")
                        nc.vector.tensor_tensor(t1, psw, sn, ALU.mult)
                        dst = qT if which == "q" else kT
                        nc.vector.tensor_tensor(dst[:, ntsl], raw, cs, ALU.mult)
                        nc.vector.tensor_tensor(dst[:, ntsl], dst[:, ntsl], t1, ALU.add)

            # ---- phase 2: transpose v to token-major, append ones column ----
            for h in range(HPC):
                nc.sync.dma_start(out=va3[h][:, :, D], in_=vones_t[:, :])
            with tc.tile_pool(name="p2ps", bufs=2, space="PSUM") as p2ps:
                for mt in range(MTILES):
                    pst = p2ps.tile([128, 128], F16, tag="tr")
                    nc.tensor.transpose(pst, vT[:, mt * 128 : (mt + 1) * 128], ident_t)
                    for h in range(HPC):
                        nc.vector.tensor_copy(va3[h][:, mt, 0:D], pst[:, h * D : (h + 1) * D])

            # ---- phase 3: attention, both heads interleaved per query-quarter,
            # ----          out-projection chunk interleaved per quarter
            with (
                tc.tile_pool(name="p3o", bufs=2, space="PSUM") as p3o,
                tc.tile_pool(name="p3s", bufs=2, space="PSUM") as p3s,
                tc.tile_pool(name="p3sb", bufs=4) as p3sb,
                tc.tile_pool(name="p3misc", bufs=2) as p3misc,
                tc.tile_pool(name="p3out", bufs=2) as p3out,
            ):
                for nq in range(NTILES):
                    nqsl = slice(nq * NT, (nq + 1) * NT)
                    o_ps = [
                        p3o.tile([D + 1, NT], F32, tag=f"o{h}", name=f"o{h}_{nq}")
                        for h in range(HPC)
                    ]
                    for mt in range(MTILES):
                        mtsl = slice(mt * 128, (mt + 1) * 128)
                        ps_s = p3s.tile([128, 2 * NT], F32, tag="s", name=f"s{nq}_{mt}")
                        for h in range(HPC):
                            hsl = slice(h * D, (h + 1) * D)
                            nc.tensor.matmul(
                                ps_s[:, h * NT : (h + 1) * NT],
                                kT[hsl, mtsl],
                                qT[hsl, nqsl],
                                start=True,
                                stop=True,
                            )
                        pT = p3sb.tile([128, 2 * NT], F16, tag="p", name=f"p{nq}_{mt}")
                        nc.scalar.activation(pT, ps_s, AF.Exp, bias=eshift_t)
                        for h in range(HPC):
                            nc.tensor.matmul(
                                o_ps[h],
                                va3[h][:, mt, :],
                                pT[:, h * NT : (h + 1) * NT],
                                start=(mt == 0),
                                stop=(mt == MTILES - 1),
                            )
                    # per-quarter epilogue: normalize rows 0..63 by row 64 (softmax sum)
                    for h in range(HPC):
                        hsl = slice(h * D, (h + 1) * D)
                        sums = p3misc.tile([1, NT], F32R, tag="sums", name=f"sm{h}_{nq}")
                        nc.scalar.copy(sums, o_ps[h][D : D + 1, :])
                        rb = p3s.tile([D, NT], F32, tag="s", name=f"rb{h}_{nq}")
                        nc.tensor.matmul(rb, ones_t, sums, start=True, stop=True)
                        rs = p3misc.tile([D, NT], F32, tag="rs", name=f"rs{h}_{nq}")
                        nc.vector.reciprocal_approx_fast(rs, rb)
                        nc.vector.tensor_tensor(oT[hsl, nqsl], o_ps[h][0:D, :], rs, ALU.mult)
                        nc.vector.tensor_scalar_add(oT[hsl, nqsl], oT[hsl, nqsl], bv_t[hsl, :])
                    # out-projection for this quarter (overlaps next quarter's attention)
                    for j in range(KC):
                        po = p3s.tile([128, NT], F32, tag="s", name=f"po{j}_{nq}")
                        nc.tensor.matmul(
                            po,
                            wp_sb[:, j * 128 : (j + 1) * 128],
                            oT[:, nqsl],
                            start=True,
                            stop=True,
                        )
                        osb = p3out.tile([128, NT], F32, tag="osb", name=f"osb{j}_{nq}")
                        nc.vector.tensor_copy(osb, po)
                        nc.sync.dma_start(out=out_part[j * 128 : (j + 1) * 128, nqsl], in_=osb)

    nc.compile()
    return nc


def _host_prep(x, rope, class_mask, w_q, b_q, w_k, w_v, b_v, w_proj):
    """Build per-core input maps. All heavy math stays on device."""
    F16 = np.float16
    x2 = np.ascontiguousarray(x.reshape(N, C).astype(np.float32))
    xT = np.ascontiguousarray(x2.T.astype(F16))

    cm = np.asarray(class_mask).reshape(N).astype(bool)
    idx = np.clip(np.cumsum(~cm) - 1, 0, rope.shape[0] - 1)
    sin_m = np.asarray(rope[:, :D], dtype=np.float32)
    cos_m = np.asarray(rope[:, D:], dtype=np.float32)
    sin = np.where(cm[:, None], 0.0, sin_m[idx]).astype(np.float32)  # [N, D]
    cos = np.where(cm[:, None], 1.0, cos_m[idx]).astype(np.float32)
    cosT = np.ascontiguousarray(cos.T)  # [D, N]
    sinT = np.ascontiguousarray(sin.T)
    # sign-fold for the pair-swap trick:
    #   roped[2i]   = q[2i]*cos[2i]   - q[2i+1]*sin[2i]
    #   roped[2i+1] = q[2i+1]*cos[2i+1] + q[2i]*sin[2i+1]
    sgn = np.where(np.arange(D) % 2 == 0, -1.0, 1.0).astype(np.float32)
    sinT_s = sinT * sgn[:, None]
    cos2 = np.concatenate([cosT, cosT], axis=0)  # [128, N] (2 head blocks)
    sin2 = np.concatenate([sinT_s, sinT_s], axis=0)
    scale = np.float32(D ** -0.5)
    cosq = (cos2 * scale).astype(F16)
    sinq = (sin2 * scale).astype(F16)
    cosk = cos2.astype(F16)
    sink = sin2.astype(F16)

    pm = np.zeros((128, 128), dtype=F16)
    for i in range(64):
        pm[2 * i + 1, 2 * i] = 1.0
        pm[2 * i, 2 * i + 1] = 1.0
    ident = np.eye(128, dtype=F16)
    ones64 = np.ones((1, D), dtype=np.float32)
    vones = np.ones((128, MTILES), dtype=F16)

    w_q = np.asarray(w_q, dtype=np.float32)
    w_k = np.asarray(w_k, dtype=np.float32)
    w_v = np.asarray(w_v, dtype=np.float32)
    w_proj = np.asarray(w_proj, dtype=np.float32)
    b_q = np.asarray(b_q, dtype=np.float32)
    b_v = np.asarray(b_v, dtype=np.float32)

    in_maps = []
    for c in range(NCORES):
        csl = slice(c * DH, (c + 1) * DH)
        in_maps.append(
            {
                "xT": xT,
                "wq": np.ascontiguousarray(w_q[:, csl].astype(F16)),
                "wk": np.ascontiguousarray(w_k[:, csl].astype(F16)),
                "wv": np.ascontiguousarray(w_v[:, csl].astype(F16)),
                "wp": np.ascontiguousarray(w_proj[csl, :].astype(F16)),
                "bq": np.ascontiguousarray(b_q[csl].reshape(DH, 1)),
                "bv": np.ascontiguousarray(b_v[csl].reshape(DH, 1)),
                "cosq": cosq,
                "sinq": sinq,
                "cosk": cosk,
                "sink": sink,
                "perm": pm,
                "ident": ident,
                "ones64": ones64,
                "vones": vones,
            }
        )
    return in_maps


def _get_prog():
    global _PROG
    if _PROG is None:
        _PROG = _build_program()
    return _PROG


def kernel(x, rope, class_mask, w_q, b_q, w_k, w_v, b_v, w_proj, b_proj, _trace=False):
    from concourse.bass_utils import run_bass_kernel_spmd

    nc = _get_prog()
    in_maps = _host_prep(x, rope, class_mask, w_q, b_q, w_k, w_v, b_v, w_proj)
    res = run_bass_kernel_spmd(nc, in_maps, core_ids=list(range(NCORES)), trace=_trace)
    acc = np.zeros((C, N), dtype=np.float64)
    for c in range(NCORES):
        acc += res.results[c]["out_part"]
    out = acc.T.astype(np.float32) + np.asarray(b_proj, dtype=np.float32)[None, :]
    out = out.reshape(1, N, C)
    if _trace:
        return out, res
    return out


# revision 9
# speedup vs baseline: 1.5044x; 1.1774x over previous
"""EvaAttention on 8 Trainium2 NeuronCores — head-parallel tensor parallelism.

Per core c (heads 2c, 2c+1):
  - q/k/v projections with column-sliced weights, dh-major layout
    (qT/kT/vT : [128 = 2 heads x 64 dims, 2048 tokens]), fp16 matmuls with
    fp32 PSUM accumulation (fp16 streams 1 cycle/row like bf16 — ~131ns per
    N=512 matmul vs ~600ns for fp32r self-loading — at 4x finer mantissa).
  - Conditional RoPE applied via host-precomputed transposed sin/cos tables
    (identity rows at class-token positions) + a pair-swap permutation matmul.
  - Attention with transposed scores (sT [keys, queries]) so softmax's
    reduction lands on the PE: exp(s - 12) on ScalarE (constant shift keeps
    exp within fp16 range; softmax is shift-invariant), softmax sums via a
    ones column appended to the V stationary operand, normalization after AV
    with reciprocal_approx_fast. Both heads processed together per
    (query-quarter, key-tile) with row-group-packed QK^T so the PE never
    idles long enough for HAM to re-throttle.
  - Row-sliced out-projection partial interleaved per query-quarter (hides
    the output DMA under the next quarter's attention); host sums the 8
    partials (the tensor-parallel all-reduce at unshard time) + b_proj.
"""

import numpy as np

N = 2048
C = 1024
H = 16
D = 64  # head dim
NCORES = 8
HPC = H // NCORES  # heads per core = 2
DH = HPC * D  # per-core channel slice = 128
NT = 512
NTILES = N // NT  # 4
MT = 128
MTILES = N // MT  # 16
KC = C // 128  # 8 contraction chunks
ESHIFT = -12.0  # exp(s + ESHIFT): keeps exp in fp16 range; cancels in softmax

_PROG = None


def _build_program():
    import concourse.bacc as bacc
    import concourse.mybir as mybir
    import concourse.tile as tile

    F32R = mybir.dt.float32r
    F32 = mybir.dt.float32
    F16 = mybir.dt.float16
    AF = mybir.ActivationFunctionType
    ALU = mybir.AluOpType

    nc = bacc.Bacc()

    xT = nc.dram_tensor("xT", [C, N], F16, kind="ExternalInput")
    wq = nc.dram_tensor("wq", [C, DH], F16, kind="ExternalInput")
    wk = nc.dram_tensor("wk", [C, DH], F16, kind="ExternalInput")
    wv = nc.dram_tensor("wv", [C, DH], F16, kind="ExternalInput")
    wp = nc.dram_tensor("wp", [DH, C], F16, kind="ExternalInput")
    bq = nc.dram_tensor("bq", [DH, 1], F32, kind="ExternalInput")
    bv = nc.dram_tensor("bv", [DH, 1], F32, kind="ExternalInput")
    cosq = nc.dram_tensor("cosq", [DH, N], F16, kind="ExternalInput")
    sinq = nc.dram_tensor("sinq", [DH, N], F16, kind="ExternalInput")
    cosk = nc.dram_tensor("cosk", [DH, N], F16, kind="ExternalInput")
    sink = nc.dram_tensor("sink", [DH, N], F16, kind="ExternalInput")
    perm = nc.dram_tensor("perm", [128, 128], F16, kind="ExternalInput")
    ident = nc.dram_tensor("ident", [128, 128], F16, kind="ExternalInput")
    ones64 = nc.dram_tensor("ones64", [1, D], F32R, kind="ExternalInput")
    vones = nc.dram_tensor("vones", [128, MTILES], F16, kind="ExternalInput")
    out_part = nc.dram_tensor("out_part", [C, N], F32, kind="ExternalOutput")

    with tile.TileContext(nc) as tc, nc.allow_low_precision(reason="fp16 intermediates, fp32 accumulate"):
        with (
            tc.tile_pool(name="const", bufs=1) as const,
            tc.tile_pool(name="work", bufs=1) as work,
        ):
            # ---- small constants first (single 3D-AP DMAs keep the issue queue short) ----
            wq_sb = const.tile([128, C], F16)
            wk_sb = const.tile([128, C], F16)
            wv_sb = const.tile([128, C], F16)
            for w_sb, w_d in ((wq_sb, wq), (wk_sb, wk), (wv_sb, wv)):
                nc.sync.dma_start(
                    out=w_sb.rearrange("p (kc d) -> p kc d", d=DH),
                    in_=w_d.rearrange("(kc p) d -> p kc d", p=128),
                )
            bq_t = const.tile([DH, 1], F32)
            bv_t = const.tile([DH, 1], F32)
            nc.sync.dma_start(out=bq_t, in_=bq[:, :])
            nc.sync.dma_start(out=bv_t, in_=bv[:, :])
            perm_t = const.tile([128, 128], F16)
            ident_t = const.tile([128, 128], F16)
            nc.sync.dma_start(out=perm_t, in_=perm[:, :])
            nc.sync.dma_start(out=ident_t, in_=ident[:, :])
            ones_t = const.tile([1, D], F32R)
            nc.sync.dma_start(out=ones_t, in_=ones64[:, :])
            vones_t = const.tile([128, MTILES], F16)
            nc.sync.dma_start(out=vones_t, in_=vones[:, :])
            wp_sb = const.tile([128, C], F16)
            nc.sync.dma_start(out=wp_sb, in_=wp[:, :])
            eshift_t = const.tile([128, 1], F32)
            nc.vector.memset(eshift_t, ESHIFT)

            # ---- x^T: one DMA per 128-channel chunk ----
            xts = [const.tile([128, N], F16, name=f"xts{kc}", tag=f"xts{kc}") for kc in range(KC)]
            for kc in range(KC):
                eng = nc.sync if kc % 2 == 0 else nc.gpsimd
                eng.dma_start(out=xts[kc], in_=xT[kc * 128 : (kc + 1) * 128, :])

            # ---- persistent work tiles ----
            qT = work.tile([DH, N], F16)  # roped, pre-scaled by d^-0.5
            kT = work.tile([DH, N], F16)  # roped
            vT = work.tile([DH, N], F16)
            va = [work.tile([128, MTILES * (D + 1)], F16, name=f"va{h}", tag=f"va{h}") for h in range(HPC)]
            va3 = [v.rearrange("p (t e) -> p t e", e=D + 1) for v in va]
            oT = work.tile([DH, N], F16)  # normalized per-head outputs (+ b_v)

            # ---- phase 1: projections + rope ----
            with (
                tc.tile_pool(name="p1sb", bufs=2) as p1sb,
                tc.tile_pool(name="p1ps", bufs=3, space="PSUM") as p1ps,
                tc.tile_pool(name="p1sw", bufs=2, space="PSUM") as p1sw,
            ):
                for nt in range(NTILES):
                    ntsl = slice(nt * NT, (nt + 1) * NT)
                    # v projection (no rope)
                    psv = p1ps.tile([128, NT], F32, tag="proj")
                    for kc in range(KC):
                        nc.tensor.matmul(
                            psv,
                            wv_sb[:, kc * 128 : (kc + 1) * 128],
                            xts[kc][:, ntsl],
                            start=(kc == 0),
                            stop=(kc == KC - 1),
                        )
                    nc.scalar.copy(vT[:, ntsl], psv)
                    # q and k projections + rope
                    for which in ("q", "k"):
                        ps = p1ps.tile([128, NT], F32, tag="proj", name=f"ps_{which}{nt}")
                        w_sb = wq_sb if which == "q" else wk_sb
                        for kc in range(KC):
                            nc.tensor.matmul(
                                ps,
                                w_sb[:, kc * 128 : (kc + 1) * 128],
                                xts[kc][:, ntsl],
                                start=(kc == 0),
                                stop=(kc == KC - 1),
                            )
                        raw = p1sb.tile([128, NT], F16, tag="raw", name=f"raw_{which}{nt}")
                        if which == "q":
                            nc.scalar.activation(raw, ps, AF.Identity, bias=bq_t)
                        else:
                            nc.scalar.copy(raw, ps)
                        # pair-swap via permutation matmul
                        psw = p1sw.tile([128, NT], F32, tag="swap", name=f"psw_{which}{nt}")
                        nc.tensor.matmul(psw, perm_t, raw, start=True, stop=True)
                        cos_d = cosq if which == "q" else cosk
                        sin_d = sinq if which == "q" else sink
                        cs = p1sb.tile([128, NT], F16, tag="cs", name=f"cs_{which}{nt}")
                        sn = p1sb.tile([128, NT], F16, tag="sn", name=f"sn_{which}{nt}")
                        nc.gpsimd.dma_start(out=cs, in_=cos_d[:, ntsl])
                        nc.gpsimd.dma_start(out=sn, in_=sin_d[:, ntsl])
                        t1 = p1sb.tile([128, NT], F32, tag="t1", name=f"t1_{which}{nt}")
                        nc.vector.tensor_tensor(t1, psw, sn, ALU.mult)
                        dst = qT if which == "q" else kT
                        nc.vector.tensor_tensor(dst[:, ntsl], raw, cs, ALU.mult)
                        nc.vector.tensor_tensor(dst[:, ntsl], dst[:, ntsl], t1, ALU.add)

            # ---- phase 2: transpose v to token-major, append ones column ----
            for h in range(HPC):
                nc.sync.dma_start(out=va3[h][:, :, D], in_=vones_t[:, :])
            with tc.tile_pool(name="p2ps", bufs=2, space="PSUM") as p2ps:
                for mt in range(MTILES):
                    pst = p2ps.tile([128, 128], F16, tag="tr")
                    nc.tensor.transpose(pst, vT[:, mt * 128 : (mt + 1) * 128], ident_t)
                    for h in range(HPC):
                        nc.vector.tensor_copy(va3[h][:, mt, 0:D], pst[:, h * D : (h + 1) * D])

            # ---- phase 3: attention, both heads interleaved per query-quarter,
            # ----          out-projection chunk interleaved per quarter
            with (
                tc.tile_pool(name="p3o", bufs=1, space="PSUM") as p3o,
                tc.tile_pool(name="p3s", bufs=2, space="PSUM") as p3s,
                tc.tile_pool(name="p3po", bufs=2, space="PSUM") as p3po,
                tc.tile_pool(name="p3sb", bufs=4) as p3sb,
                tc.tile_pool(name="p3misc", bufs=2) as p3misc,
                tc.tile_pool(name="p3out", bufs=2) as p3out,
            ):
                for nq in range(NTILES):
                    nqsl = slice(nq * NT, (nq + 1) * NT)
                    o_ps = [
                        p3o.tile([D + 1, NT], F32, tag=f"o{h}", name=f"o{h}_{nq}")
                        for h in range(HPC)
                    ]
                    for mt in range(MTILES):
                        mtsl = slice(mt * 128, (mt + 1) * 128)
                        ps_s = p3s.tile([128, 2 * NT], F32, tag="s", name=f"s{nq}_{mt}")
                        for h in range(HPC):
                            hsl = slice(h * D, (h + 1) * D)
                            nc.tensor.matmul(
                                ps_s[:, h * NT : (h + 1) * NT],
                                kT[hsl, mtsl],
                                qT[hsl, nqsl],
                                start=True,
                                stop=True,
                            )
                        pT = p3sb.tile([128, 2 * NT], F16, tag="p", name=f"p{nq}_{mt}")
                        nc.scalar.activation(pT, ps_s, AF.Exp, bias=eshift_t)
                        for h in range(HPC):
                            nc.tensor.matmul(
                                o_ps[h],
                                va3[h][:, mt, :],
                                pT[:, h * NT : (h + 1) * NT],
                                start=(mt == 0),
                                stop=(mt == MTILES - 1),
                            )
                    # per-quarter epilogue: normalize rows 0..63 by row 64 (softmax sum)
                    for h in range(HPC):
                        hsl = slice(h * D, (h + 1) * D)
                        sums = p3misc.tile([1, NT], F32R, tag="sums", name=f"sm{h}_{nq}")
                        nc.scalar.copy(sums, o_ps[h][D : D + 1, :])
                        rb = p3po.tile([D, NT], F32, tag="po", name=f"rb{h}_{nq}")
                        nc.tensor.matmul(rb, ones_t, sums, start=True, stop=True)
                        rs = p3misc.tile([D, NT], F32, tag="rs", name=f"rs{h}_{nq}")
                        nc.vector.reciprocal_approx_fast(rs, rb)
                        nc.vector.tensor_tensor(oT[hsl, nqsl], o_ps[h][0:D, :], rs, ALU.mult)
                        nc.vector.tensor_scalar_add(oT[hsl, nqsl], oT[hsl, nqsl], bv_t[hsl, :])
                    # out-projection for this quarter (overlaps next quarter's attention)
                    for j in range(KC):
                        po = p3po.tile([128, NT], F32, tag="po", name=f"po{j}_{nq}")
                        nc.tensor.matmul(
                            po,
                            wp_sb[:, j * 128 : (j + 1) * 128],
                            oT[:, nqsl],
                            start=True,
                            stop=True,
                        )
                        osb = p3out.tile([128, NT], F32, tag="osb", name=f"osb{j}_{nq}")
                        if j % 2 == 0:
                            nc.vector.tensor_copy(osb, po)
                        else:
                            nc.scalar.copy(osb, po)
                        nc.sync.dma_start(out=out_part[j * 128 : (j + 1) * 128, nqsl], in_=osb)

    nc.compile()
    return nc


def _host_prep(x, rope, class_mask, w_q, b_q, w_k, w_v, b_v, w_proj):
    """Build per-core input maps. All heavy math stays on device."""
    F16 = np.float16
    x2 = np.ascontiguousarray(x.reshape(N, C).astype(np.float32))
    xT = np.ascontiguousarray(x2.T.astype(F16))

    cm = np.asarray(class_mask).reshape(N).astype(bool)
    idx = np.clip(np.cumsum(~cm) - 1, 0, rope.shape[0] - 1)
    sin_m = np.asarray(rope[:, :D], dtype=np.float32)
    cos_m = np.asarray(rope[:, D:], dtype=np.float32)
    sin = np.where(cm[:, None], 0.0, sin_m[idx]).astype(np.float32)  # [N, D]
    cos = np.where(cm[:, None], 1.0, cos_m[idx]).astype(np.float32)
    cosT = np.ascontiguousarray(cos.T)  # [D, N]
    sinT = np.ascontiguousarray(sin.T)
    # sign-fold for the pair-swap trick:
    #   roped[2i]   = q[2i]*cos[2i]   - q[2i+1]*sin[2i]
    #   roped[2i+1] = q[2i+1]*cos[2i+1] + q[2i]*sin[2i+1]
    sgn = np.where(np.arange(D) % 2 == 0, -1.0, 1.0).astype(np.float32)
    sinT_s = sinT * sgn[:, None]
    cos2 = np.concatenate([cosT, cosT], axis=0)  # [128, N] (2 head blocks)
    sin2 = np.concatenate([sinT_s, sinT_s], axis=0)
    scale = np.float32(D ** -0.5)
    cosq = (cos2 * scale).astype(F16)
    sinq = (sin2 * scale).astype(F16)
    cosk = cos2.astype(F16)
    sink = sin2.astype(F16)

    pm = np.zeros((128, 128), dtype=F16)
    for i in range(64):
        pm[2 * i + 1, 2 * i] = 1.0
        pm[2 * i, 2 * i + 1] = 1.0
    ident = np.eye(128, dtype=F16)
    ones64 = np.ones((1, D), dtype=np.float32)
    vones = np.ones((128, MTILES), dtype=F16)

    w_q = np.asarray(w_q, dtype=np.float32)
    w_k = np.asarray(w_k, dtype=np.float32)
    w_v = np.asarray(w_v, dtype=np.float32)
    w_proj = np.asarray(w_proj, dtype=np.float32)
    b_q = np.asarray(b_q, dtype=np.float32)
    b_v = np.asarray(b_v, dtype=np.float32)

    in_maps = []
    for c in range(NCORES):
        csl = slice(c * DH, (c + 1) * DH)
        in_maps.append(
            {
                "xT": xT,
                "wq": np.ascontiguousarray(w_q[:, csl].astype(F16)),
                "wk": np.ascontiguousarray(w_k[:, csl].astype(F16)),
                "wv": np.ascontiguousarray(w_v[:, csl].astype(F16)),
                "wp": np.ascontiguousarray(w_proj[csl, :].astype(F16)),
                "bq": np.ascontiguousarray(b_q[csl].reshape(DH, 1)),
                "bv": np.ascontiguousarray(b_v[csl].reshape(DH, 1)),
                "cosq": cosq,
                "sinq": sinq,
                "cosk": cosk,
                "sink": sink,
                "perm": pm,
                "ident": ident,
                "ones64": ones64,
                "vones": vones,
            }
        )
    return in_maps


def _get_prog():
    global _PROG
    if _PROG is None:
        _PROG = _build_program()
    return _PROG


def kernel(x, rope, class_mask, w_q, b_q, w_k, w_v, b_v, w_proj, b_proj, _trace=False):
    from concourse.bass_utils import run_bass_kernel_spmd

    nc = _get_prog()
    in_maps = _host_prep(x, rope, class_mask, w_q, b_q, w_k, w_v, b_v, w_proj)
    res = run_bass_kernel_spmd(nc, in_maps, core_ids=list(range(NCORES)), trace=_trace)
    acc = np.zeros((C, N), dtype=np.float64)
    for c in range(NCORES):
        acc += res.results[c]["out_part"]
    out = acc.T.astype(np.float32) + np.asarray(b_proj, dtype=np.float32)[None, :]
    out = out.reshape(1, N, C)
    if _trace:
        return out, res
    return out
